# revision 1
# baseline (speedup 1.0000x reference)
"""Trainium2 Bass kernel for nn_Net_69114613727316 (RGCN message passing).

Self-contained: kernel(**inputs) -> np.ndarray [100000] float32.

Math (exploiting num_bases=1): w[r] = att[r,0] * basis, so
    msg_e = att[etype_e] * (x[src_e] @ basis)
    agg_d = ((sum_e v_e * x[src_e]) @ basis),  v_e = att_l[etype_e]/max(cnt_d,1)
    out = relu(agg + x @ root + bias)
The per-edge GEMMs of the reference collapse into a weighted scatter-add
(done as one-hot matmuls into PSUM per 128-dst band) plus one dense GEMM
per band.

Sharding: node space is permuted so core c owns rows
[c*25088, (c+1)*25088) = [12544 var slots | 12544 con slots] (12500 real
each + pad).  Each layer's full x is replicated via AllGather so any core
can gather arbitrary src rows; each core computes only its own rows.
"""
import numpy as np

import concourse.bass as bass
import concourse.bacc as bacc
import concourse.tile as tile
import concourse.mybir as mybir
from concourse.masks import make_identity

F32 = mybir.dt.float32
I32 = mybir.dt.int32
D = 128
P = 128

N_VAR = 100000
N_CON = 100000
N_EDGES = 640000
N_CORES = 8
GROUP_BANDS = 8

NV_CORE = N_VAR // N_CORES          # 12500
NC_CORE = N_CON // N_CORES          # 12500
NV_SLOT = ((NV_CORE + P - 1) // P) * P   # 12544
NC_SLOT = ((NC_CORE + P - 1) // P) * P   # 12544
PER_CORE = NV_SLOT + NC_SLOT        # 25088
N_PAD = N_CORES * PER_CORE          # 200704
BANDS = PER_CORE // P               # 196


def _preprocess(inputs):
    vf = np.ascontiguousarray(np.asarray(inputs["var_node_features"], np.float32))
    cf = np.ascontiguousarray(np.asarray(inputs["con_node_features"], np.float32))
    ei = np.asarray(inputs["edge_index"])
    et = np.asarray(inputs["edge_types"]).astype(np.int64)
    assoc_var = np.asarray(inputs["assoc_var"])
    assoc_con = np.asarray(inputs["assoc_con"])
    assert (assoc_var == np.arange(N_VAR)).all()
    assert (assoc_con == N_VAR + np.arange(N_CON)).all()

    src = ei[0].astype(np.int64)
    dst = ei[1].astype(np.int64)

    def pi(node):
        isv = node < N_VAR
        k = np.where(isv, node, node - N_VAR)
        cdiv = np.where(isv, NV_CORE, NC_CORE)
        return (k // cdiv) * PER_CORE + np.where(isv, 0, NV_SLOT) + k % cdiv

    psrc = pi(src)
    pdst = pi(dst)
    cnt = np.bincount(pdst, minlength=N_PAD).astype(np.float32)
    invc = 1.0 / np.maximum(cnt, 1.0)
    atts = [np.asarray(inputs[f"att{l}"], np.float32)[:, 0] for l in (1, 2, 3)]

    core = pdst // PER_CORE
    band = (pdst % PER_CORE) // P
    dloc = (pdst % P).astype(np.float32)

    seg = core * BANDS + band
    counts = np.bincount(seg, minlength=N_CORES * BANDS).reshape(N_CORES, BANDS)
    tpb = (np.ceil(counts.max(axis=0) / P)).astype(np.int64)
    T_total = int(tpb.sum())
    toff = np.concatenate([[0], np.cumsum(tpb)[:-1]]).astype(np.int64)

    order = np.argsort(seg, kind="stable")
    seg_s = seg[order]
    run_starts = np.concatenate(
        [[0], np.cumsum(np.bincount(seg_s, minlength=N_CORES * BANDS))[:-1]])
    rank = np.arange(len(order)) - run_starts[seg_s]
    core_s = core[order]
    pos = (rank % P) * T_total + toff[band[order]] + rank // P

    E_slots = T_total * P
    src_arr = np.zeros((N_CORES, E_slots), np.int32)
    dl_arr = np.zeros((N_CORES, E_slots), np.float32)
    vw_arr = np.zeros((3, N_CORES, E_slots), np.float32)
    src_arr[core_s, pos] = psrc[order].astype(np.int32)
    dl_arr[core_s, pos] = dloc[order]
    for li in range(3):
        vw_arr[li, core_s, pos] = atts[li][et[order]] * invc[pdst[order]]

    in_maps = []
    for c in range(N_CORES):
        vfeat = np.zeros((NV_SLOT, 2), np.float32)
        vfeat[:NV_CORE] = vf[c * NV_CORE:(c + 1) * NV_CORE]
        cfeat = np.zeros((NC_SLOT, 2), np.float32)
        cfeat[:NC_CORE] = cf[c * NC_CORE:(c + 1) * NC_CORE]
        m = {
            "vfeat": vfeat,
            "cfeat": cfeat,
            "src_idx": src_arr[c].reshape(P, T_total),
            "dstloc": dl_arr[c].reshape(P, T_total),
            "vw1": vw_arr[0, c].reshape(P, T_total),
            "vw2": vw_arr[1, c].reshape(P, T_total),
            "vw3": vw_arr[2, c].reshape(P, T_total),
            "fc1_w": np.asarray(inputs["fc1_w"], np.float32),
            "fc1_b": np.asarray(inputs["fc1_b"], np.float32),
            "fc4_w": np.asarray(inputs["fc4_w"], np.float32),
            "fc4_b": np.asarray(inputs["fc4_b"], np.float32).reshape(1, 1),
        }
        for t in ("var", "con"):
            for wname in ("w1", "b1", "w2", "b2"):
                m[f"{t}_{wname}"] = np.asarray(inputs[f"{t}_{wname}"], np.float32)
        for l in (1, 2, 3):
            m[f"basis{l}"] = np.asarray(inputs[f"basis{l}"], np.float32).reshape(D, D)
            m[f"root{l}"] = np.asarray(inputs[f"root{l}"], np.float32)
            m[f"bias{l}"] = np.asarray(inputs[f"bias{l}"], np.float32)
        in_maps.append(m)
    return in_maps, [int(t) for t in tpb]


def _build_program(tpb):
    nb = BANDS
    T_total = sum(tpb)
    toff = [0]
    for t in tpb[:-1]:
        toff.append(toff[-1] + t)

    nc = bacc.Bacc("TRN2", target_bir_lowering=False, debug=False,
                   num_devices=N_CORES)

    def inp(name, shape, dtype=F32):
        return nc.dram_tensor(name, shape, dtype, kind="ExternalInput")

    vfeat = inp("vfeat", [NV_SLOT, 2])
    cfeat = inp("cfeat", [NC_SLOT, 2])
    src_idx = inp("src_idx", [P, T_total], I32)
    dstloc = inp("dstloc", [P, T_total])
    vws = {l: inp(f"vw{l}", [P, T_total]) for l in (1, 2, 3)}
    mlp_w = {}
    for t in ("var", "con"):
        mlp_w[t] = (inp(f"{t}_w1", [2, D]), inp(f"{t}_b1", [D]),
                    inp(f"{t}_w2", [D, D]), inp(f"{t}_b2", [D]))
    rg_w = {l: (inp(f"basis{l}", [D, D]), inp(f"root{l}", [D, D]),
                inp(f"bias{l}", [D])) for l in (1, 2, 3)}
    fc1_w = inp("fc1_w", [4 * D, D])
    fc1_b = inp("fc1_b", [D])
    fc4_w = inp("fc4_w", [D, 1])
    fc4_b = inp("fc4_b", [1, 1])
    y_out = nc.dram_tensor("y_out", [NV_SLOT], F32, kind="ExternalOutput")

    x_full = [nc.dram_tensor(f"x{i}_full", [N_PAD, D], F32, kind="Internal",
                             addr_space="Shared") for i in range(3)]
    ag_in = [nc.dram_tensor(f"ag_in{i}", [PER_CORE, D], F32, kind="Internal")
             for i in range(3)]
    xT_own = [nc.dram_tensor(f"xT{i}_own", [D, PER_CORE], F32, kind="Internal")
              for i in range(4)]

    rgroups = [list(range(N_CORES))]

    with tile.TileContext(nc) as tc:
        with tc.tile_pool(name="wp", bufs=1) as wp:
            iota_t = wp.tile([P, P], F32)
            nc.gpsimd.iota(iota_t[:], pattern=[[1, P]], base=0,
                           channel_multiplier=0,
                           allow_small_or_imprecise_dtypes=True)
            ident = wp.tile([P, P], F32)
            make_identity(nc, ident[:])

            # ---------- phase A: input MLPs -> x0 ----------
            with tc.tile_pool(name="pa_sb", bufs=3) as sp, \
                 tc.tile_pool(name="pa_ps", bufs=2, space="PSUM") as pp:
                for ttype, feat, slot0, nslot in (
                    ("var", vfeat, 0, NV_SLOT),
                    ("con", cfeat, NV_SLOT, NC_SLOT),
                ):
                    w1, b1, w2, b2 = mlp_w[ttype]
                    w1s = wp.tile([2, D], F32, name=f"w1s_{ttype}")
                    nc.sync.dma_start(w1s[:], w1[:])
                    b1s = wp.tile([P, 1], F32, name=f"b1s_{ttype}")
                    nc.sync.dma_start(b1s[:], b1.rearrange("(p one) -> p one", one=1))
                    w2s = wp.tile([D, D], F32, name=f"w2s_{ttype}")
                    nc.sync.dma_start(w2s[:], w2[:])
                    b2s = wp.tile([P, 1], F32, name=f"b2s_{ttype}")
                    nc.sync.dma_start(b2s[:], b2.rearrange("(p one) -> p one", one=1))
                    for j in range(nslot // P):
                        col0 = slot0 + j * P
                        ft = sp.tile([2, P], F32, name="ft")
                        nc.sync.dma_start(
                            ft[:], feat[j * P:(j + 1) * P, :].rearrange("n d -> d n"))
                        p1 = pp.tile([P, P], F32, name="p1", space="PSUM")
                        nc.tensor.matmul(p1[:], lhsT=w1s[:], rhs=ft[:],
                                         start=True, stop=True)
                        h1 = sp.tile([P, P], F32, name="h1")
                        nc.scalar.activation(h1[:], p1[:],
                                             mybir.ActivationFunctionType.Relu,
                                             bias=b1s[:, :1])
                        p2 = pp.tile([P, P], F32, name="p2", space="PSUM")
                        nc.tensor.matmul(p2[:], lhsT=w2s[:], rhs=h1[:],
                                         start=True, stop=True)
                        x0t = sp.tile([P, P], F32, name="x0t")
                        nc.vector.tensor_scalar(x0t[:], p2[:], b2s[:, :1], None,
                                                op0=mybir.AluOpType.add)
                        nc.sync.dma_start(xT_own[0][:, col0:col0 + P], x0t[:])
                        tp = pp.tile([P, P], F32, name="tp", space="PSUM")
                        nc.tensor.transpose(tp[:], x0t[:], ident[:])
                        x0r = sp.tile([P, P], F32, name="x0r")
                        nc.scalar.activation(x0r[:], tp[:],
                                             mybir.ActivationFunctionType.Copy)
                        nc.sync.dma_start(ag_in[0][col0:col0 + P, :], x0r[:])
            nc.gpsimd.collective_compute(
                "AllGather", mybir.AluOpType.bypass, replica_groups=rgroups,
                ins=[ag_in[0][:]], outs=[x_full[0][:]])

            # ---------- phases B: 3 RGCN layers ----------
            for l in (1, 2, 3):
                basis, root, bias = rg_w[l]
                Bs = wp.tile([D, D], F32, name=f"Bs_{l}")
                nc.sync.dma_start(Bs[:], basis[:])
                Rs = wp.tile([D, D], F32, name=f"Rs_{l}")
                nc.sync.dma_start(Rs[:], root[:])
                bs = wp.tile([P, 1], F32, name=f"bs_{l}")
                nc.sync.dma_start(bs[:], bias.rearrange("(p one) -> p one", one=1))
                xcur = x_full[l - 1]
                xTc = xT_own[l - 1]
                xTn = xT_own[l]
                vw = vws[l]
                max_tg = max(
                    (sum(tpb[g:g + GROUP_BANDS])
                     for g in range(0, nb, GROUP_BANDS)), default=1)
                with tc.tile_pool(name=f"l{l}_g", bufs=2) as gp, \
                     tc.tile_pool(name=f"l{l}_sb", bufs=3) as sp, \
                     tc.tile_pool(name=f"l{l}_o", bufs=4) as op, \
                     tc.tile_pool(name=f"l{l}_ps", bufs=2, space="PSUM") as pp:
                    for g0 in range(0, nb, GROUP_BANDS):
                        gbands = list(range(g0, min(g0 + GROUP_BANDS, nb)))
                        tg = sum(tpb[m] for m in gbands)
                        goff = toff[g0]
                        if tg > 0:
                            idxg = gp.tile([P, max_tg], I32, name="idxg")
                            nc.sync.dma_start(idxg[:, :tg],
                                              src_idx[:, goff:goff + tg])
                            dlg = gp.tile([P, max_tg], F32, name="dlg")
                            nc.sync.dma_start(dlg[:, :tg],
                                              dstloc[:, goff:goff + tg])
                            vg = gp.tile([P, max_tg], F32, name="vg")
                            nc.sync.dma_start(vg[:, :tg], vw[:, goff:goff + tg])
                            xg = gp.tile([P, max_tg * P], F32, name="xg")
                            for t in range(tg):
                                nc.gpsimd.indirect_dma_start(
                                    out=xg[:, t * P:(t + 1) * P],
                                    out_offset=None,
                                    in_=xcur[:],
                                    in_offset=bass.IndirectOffsetOnAxis(
                                        ap=idxg[:, t:t + 1], axis=0))
                        for m in gbands:
                            tpbm = tpb[m]
                            st = pp.tile([P, P], F32, name="st", space="PSUM")
                            for t in range(tpbm):
                                tt = toff[m] - goff + t
                                o = op.tile([P, P], F32, name="o")
                                nc.vector.tensor_scalar(
                                    o[:], iota_t[:], dlg[:, tt:tt + 1],
                                    vg[:, tt:tt + 1],
                                    op0=mybir.AluOpType.is_equal,
                                    op1=mybir.AluOpType.mult)
                                nc.tensor.matmul(
                                    st[:], lhsT=xg[:, tt * P:(tt + 1) * P],
                                    rhs=o[:], start=(t == 0),
                                    stop=(t == tpbm - 1))
                            xtb = sp.tile([P, P], F32, name="xtb")
                            nc.sync.dma_start(xtb[:], xTc[:, m * P:(m + 1) * P])
                            p2 = pp.tile([P, P], F32, name="p2", space="PSUM")
                            if tpbm > 0:
                                sn = sp.tile([P, P], F32, name="sn")
                                nc.scalar.activation(
                                    sn[:], st[:],
                                    mybir.ActivationFunctionType.Copy)
                                nc.tensor.matmul(p2[:], lhsT=Bs[:], rhs=sn[:],
                                                 start=True, stop=False)
                                nc.tensor.matmul(p2[:], lhsT=Rs[:], rhs=xtb[:],
                                                 start=False, stop=True)
                            else:
                                nc.tensor.matmul(p2[:], lhsT=Rs[:], rhs=xtb[:],
                                                 start=True, stop=True)
                            outT = sp.tile([P, P], F32, name="outT")
                            nc.scalar.activation(
                                outT[:], p2[:],
                                mybir.ActivationFunctionType.Relu,
                                bias=bs[:, :1])
                            nc.sync.dma_start(xTn[:, m * P:(m + 1) * P], outT[:])
                            if l < 3:
                                tp2 = pp.tile([P, P], F32, name="tp2",
                                              space="PSUM")
                                nc.tensor.transpose(tp2[:], outT[:], ident[:])
                                orow = sp.tile([P, P], F32, name="orow")
                                nc.vector.tensor_copy(orow[:], tp2[:])
                                nc.sync.dma_start(
                                    ag_in[l][m * P:(m + 1) * P, :], orow[:])
                if l < 3:
                    nc.gpsimd.collective_compute(
                        "AllGather", mybir.AluOpType.bypass,
                        replica_groups=rgroups,
                        ins=[ag_in[l][:]], outs=[x_full[l][:]])

            # ---------- phase C: head MLP over own var rows ----------
            with tc.tile_pool(name="hd_sb", bufs=3) as sp, \
                 tc.tile_pool(name="hd_ps", bufs=2, space="PSUM") as pp:
                fc1c = []
                for l in range(4):
                    t = wp.tile([D, D], F32, name=f"fc1c{l}")
                    nc.sync.dma_start(t[:], fc1_w[l * D:(l + 1) * D, :])
                    fc1c.append(t)
                fb1 = wp.tile([P, 1], F32, name="fb1")
                nc.sync.dma_start(fb1[:], fc1_b.rearrange("(p one) -> p one", one=1))
                f4w = wp.tile([D, 1], F32, name="f4w")
                nc.sync.dma_start(f4w[:], fc4_w[:])
                f4b = wp.tile([1, 1], F32, name="f4b")
                nc.sync.dma_start(f4b[:], fc4_b[:])
                ones = wp.tile([1, P], F32, name="ones")
                nc.vector.memset(ones[:], 1.0)
                for j in range(NV_SLOT // P):
                    hp = pp.tile([P, P], F32, name="hp", space="PSUM")
                    for l in range(4):
                        xtl = sp.tile([P, P], F32, name="xtl")
                        nc.sync.dma_start(xtl[:],
                                          xT_own[l][:, j * P:(j + 1) * P])
                        nc.tensor.matmul(hp[:], lhsT=fc1c[l][:], rhs=xtl[:],
                                         start=(l == 0), stop=(l == 3))
                    hr = sp.tile([P, P], F32, name="hr")
                    nc.scalar.activation(hr[:], hp[:],
                                         mybir.ActivationFunctionType.Relu,
                                         bias=fb1[:, :1])
                    yp = pp.tile([P, 1], F32, name="yp", space="PSUM")
                    nc.tensor.matmul(yp[:], lhsT=hr[:], rhs=f4w[:],
                                     start=True, stop=False)
                    nc.tensor.matmul(yp[:], lhsT=ones[:], rhs=f4b[:],
                                     start=False, stop=True)
                    ys = sp.tile([P, 1], F32, name="ys")
                    nc.vector.tensor_copy(ys[:], yp[:])
                    nc.sync.dma_start(
                        y_out[j * P:(j + 1) * P].rearrange("(p one) -> p one",
                                                           one=1), ys[:])

    nc.compile()
    return nc


_CACHE = {}


def kernel(**inputs) -> np.ndarray:
    from concourse import bass_utils
    in_maps, tpb = _preprocess(inputs)
    key = tuple(tpb)
    if key not in _CACHE:
        _CACHE[key] = _build_program(tpb)
    nc = _CACHE[key]
    res = bass_utils.run_bass_kernel_spmd(
        nc, in_maps, core_ids=list(range(N_CORES)))
    ys = [res.results[c]["y_out"][:NV_CORE] for c in range(N_CORES)]
    return np.concatenate(ys, axis=0).astype(np.float32)
